# revision 2
# baseline (speedup 1.0000x reference)
"""NeighborAttention (B=4, N=4096, K=32, C=128, H=4) on 8 Trainium2 cores.

Data-parallel: the flattened (B*N) node axis is sharded across 8 cores;
the four small weight matrices are replicated. Inside each core everything
is channel-major ([row 4d+h, node-major free]):

  ET = (E*mask)^T            -> K,V of masked neighbors are exactly 0
  KT = WK' @ ET, VT = WV' @ ET, QT = (WQ'/sqrt(d)) @ XT        (PE)
  prod = KT * broadcast_j(QT)                                  (DVE)
  s_rep = Hrep @ prod        head-summed scores, replicated over d (PE)
  e = exp(s_rep)             no max-subtraction needed: |s| is small, and
                             softmax is shift-invariant               (ACT)
  z = sum_j e - (K - cnt[n]) masked j contribute exp(0)=1; host sends the
                             per-node count correction               (DVE)
  uv = e * VT;  umax = max_j uv;  usum = sum_j uv                    (DVE)
  out = (WO_mean+WO_sum)' @ (usum/z) + WO_max' @ (umax/z)            (PE)

attn sums to exactly 1, so aggr_mean == aggr_sum (within 1e-8) and the
mean/sum W_O blocks fold together on the host.
"""
import numpy as np
import concourse.bass as bass
import concourse.bacc as bacc
import concourse.mybir as mybir
from concourse import tile
from concourse.bass_utils import run_bass_kernel_spmd

F32 = mybir.dt.float32
AX = mybir.AxisListType.X
ALU = mybir.AluOpType

K = 32
C = 128
H = 4
D = 32
NCORES = 8

SUB_N = 16
SUB_COLS = SUB_N * K
CHUNK_N = 128
CHUNK_COLS = CHUNK_N * K

_NC_CACHE = {}


def _build_nc(nloc):
    assert nloc % CHUNK_N == 0
    if nloc in _NC_CACHE:
        return _NC_CACHE[nloc]
    nchunks = nloc // CHUNK_N
    nsub = CHUNK_COLS // SUB_COLS

    nc = bacc.Bacc()
    et = nc.dram_tensor("et", [C, nloc * K], F32, kind="ExternalInput")
    xt = nc.dram_tensor("xt", [C, nloc], F32, kind="ExternalInput")
    wqt = nc.dram_tensor("wqt", [C, C], F32, kind="ExternalInput")
    wkt = nc.dram_tensor("wkt", [C, C], F32, kind="ExternalInput")
    wvt = nc.dram_tensor("wvt", [C, C], F32, kind="ExternalInput")
    hrep = nc.dram_tensor("hrep", [C, C], F32, kind="ExternalInput")
    wost = nc.dram_tensor("wost", [C, C], F32, kind="ExternalInput")
    wo3t = nc.dram_tensor("wo3t", [C, C], F32, kind="ExternalInput")
    mcorr = nc.dram_tensor("mcorr", [C, nloc], F32, kind="ExternalInput")
    out = nc.dram_tensor("out", [C, nloc], F32, kind="ExternalOutput")

    with tile.TileContext(nc) as tc:
        with tc.tile_pool(name="wts", bufs=1) as wpool, \
             tc.tile_pool(name="xin", bufs=1) as xpool, \
             tc.tile_pool(name="etp", bufs=3) as etpool, \
             tc.tile_pool(name="work", bufs=5) as work, \
             tc.tile_pool(name="acc", bufs=1) as accp, \
             tc.tile_pool(name="epi", bufs=1) as epip, \
             tc.tile_pool(name="outp", bufs=1) as outp, \
             tc.tile_pool(name="pkv", bufs=5, space="PSUM") as pkv, \
             tc.tile_pool(name="psc", bufs=2, space="PSUM") as psc, \
             tc.tile_pool(name="psmall", bufs=1, space="PSUM") as psmall:

            w_q = wpool.tile([C, C], F32, tag="wq")
            w_k = wpool.tile([C, C], F32, tag="wk")
            w_v = wpool.tile([C, C], F32, tag="wv")
            w_h = wpool.tile([C, C], F32, tag="wh")
            w_os = wpool.tile([C, C], F32, tag="wos")
            w_o3 = wpool.tile([C, C], F32, tag="wo3")
            nc.sync.dma_start(w_q[:], wqt[:])
            nc.sync.dma_start(w_k[:], wkt[:])
            nc.sync.dma_start(w_v[:], wvt[:])
            nc.sync.dma_start(w_h[:], hrep[:])
            nc.sync.dma_start(w_os[:], wost[:])
            nc.sync.dma_start(w_o3[:], wo3t[:])

            xt_sb = xpool.tile([C, nloc], F32, tag="xt")
            nc.sync.dma_start(xt_sb[:], xt[:])
            mc_sb = xpool.tile([C, nloc], F32, tag="mc")
            nc.sync.dma_start(mc_sb[:], mcorr[:])

            out_sb = outp.tile([C, nloc], F32, tag="osb")

            umax_c = accp.tile([C, nloc], F32, tag="umax")
            usum_c = accp.tile([C, nloc], F32, tag="usum")
            z_c = accp.tile([C, nloc], F32, tag="zc")

            for ch in range(nchunks):
                n0 = ch * CHUNK_N
                c0 = ch * CHUNK_COLS

                et_sb = etpool.tile([C, CHUNK_COLS], F32, tag="et")
                nc.sync.dma_start(et_sb[:], et[:, c0:c0 + CHUNK_COLS])

                q_ps = psmall.tile([C, CHUNK_N], F32, tag="qo")
                nc.tensor.matmul(q_ps[:], w_q[:], xt_sb[:, n0:n0 + CHUNK_N],
                                 start=True, stop=True)
                q_sb = work.tile([C, CHUNK_N], F32, tag="qsb")
                nc.vector.tensor_copy(q_sb[:], q_ps[:])

                for s in range(nsub):
                    sc0 = s * SUB_COLS
                    snl = s * SUB_N
                    sn0 = n0 + snl
                    esl = et_sb[:, sc0:sc0 + SUB_COLS]

                    kt_ps = pkv.tile([C, SUB_COLS], F32, tag="kv")
                    nc.tensor.matmul(kt_ps[:], w_k[:], esl, start=True, stop=True)
                    vt_ps = pkv.tile([C, SUB_COLS], F32, tag="kv")
                    nc.tensor.matmul(vt_ps[:], w_v[:], esl, start=True, stop=True)

                    qb = q_sb[:, snl:snl + SUB_N].unsqueeze(2).broadcast_to(
                        (C, SUB_N, K))
                    prod = work.tile([C, SUB_COLS], F32, tag="prod")
                    nc.vector.tensor_mul(
                        prod[:].rearrange("p (n j) -> p n j", j=K),
                        kt_ps[:].rearrange("p (n j) -> p n j", j=K),
                        qb)

                    s_ps = psc.tile([C, SUB_COLS], F32, tag="srep")
                    nc.tensor.matmul(s_ps[:], w_h[:], prod[:],
                                     start=True, stop=True)

                    erep = work.tile([C, SUB_COLS], F32, tag="erep")
                    nc.scalar.activation(erep[:], s_ps[:],
                                         mybir.ActivationFunctionType.Exp)

                    uv = work.tile([C, SUB_COLS], F32, tag="uv")
                    nc.vector.tensor_mul(uv[:], erep[:], vt_ps[:])

                    uv_v = uv[:].rearrange("p (n j) -> p n j", j=K)
                    e_v = erep[:].rearrange("p (n j) -> p n j", j=K)
                    nc.vector.tensor_reduce(
                        umax_c[:, sn0:sn0 + SUB_N], uv_v, axis=AX, op=ALU.max)
                    nc.vector.tensor_reduce(
                        usum_c[:, sn0:sn0 + SUB_N], uv_v, axis=AX, op=ALU.add)
                    nc.vector.tensor_reduce(
                        z_c[:, sn0:sn0 + SUB_N], e_v, axis=AX, op=ALU.add)

            zcor = epip.tile([C, nloc], F32, tag="zcor")
            nc.vector.tensor_sub(zcor[:], z_c[:], mc_sb[:])
            # fully-masked nodes: umax/usum rows are exactly 0, so any
            # finite 1/z gives the correct 0 output — just avoid inf*0.
            nc.vector.tensor_scalar_max(zcor[:], zcor[:], 1e-20)
            rz = epip.tile([C, nloc], F32, tag="rz")
            nc.vector.reciprocal(rz[:], zcor[:])

            wsn = epip.tile([C, nloc], F32, tag="wsn")
            nc.vector.tensor_mul(wsn[:], usum_c[:], rz[:])
            mxn = epip.tile([C, nloc], F32, tag="mxn")
            nc.vector.tensor_mul(mxn[:], umax_c[:], rz[:])

            ob = min(512, nloc)
            for b0 in range(0, nloc, ob):
                o_ps = psmall.tile([C, ob], F32, tag="qo")
                nc.tensor.matmul(o_ps[:], w_os[:], wsn[:, b0:b0 + ob],
                                 start=True, stop=False)
                nc.tensor.matmul(o_ps[:], w_o3[:], mxn[:, b0:b0 + ob],
                                 start=False, stop=True)
                nc.scalar.copy(out_sb[:, b0:b0 + ob], o_ps[:])

            nc.sync.dma_start(out[:], out_sb[:])

    nc.compile()
    _NC_CACHE[nloc] = nc
    return nc


def _perm_dh(w):
    """[(h*32+d), cin] -> [cin, (4d+h)]"""
    wt = np.asarray(w).reshape(H, D, -1)
    return np.ascontiguousarray(np.transpose(wt, (2, 1, 0)).reshape(-1, H * D))


def build_nc(nloc):
    return _build_nc(nloc)


def prep_inputs(h_X, h_E, mask_attn, W_Q, W_K, W_V, W_O):
    h_X = np.asarray(h_X, dtype=np.float32)
    h_E = np.asarray(h_E, dtype=np.float32)
    mask_attn = np.asarray(mask_attn)
    W_Q = np.asarray(W_Q, dtype=np.float32)
    W_K = np.asarray(W_K, dtype=np.float32)
    W_V = np.asarray(W_V, dtype=np.float32)
    W_O = np.asarray(W_O, dtype=np.float32)

    B, N, Kn, Cin = h_E.shape
    BN = B * N
    nloc = BN // NCORES

    maskf = mask_attn.astype(np.float32)
    e_m = (h_E * maskf[..., None]).reshape(BN, Kn, Cin)
    xf = h_X.reshape(BN, -1)
    cnt = maskf.reshape(BN, Kn).sum(axis=1)

    wqt = _perm_dh(W_Q / np.sqrt(D))
    wkt = _perm_dh(W_K)
    wvt = _perm_dh(W_V)

    idx = np.arange(C)
    hh = idx % H
    hrep = (hh[:, None] == hh[None, :]).astype(np.float32)

    wos = W_O[:, :C] + W_O[:, C:2 * C]
    wo3 = W_O[:, 2 * C:]
    wost = np.ascontiguousarray(
        wos.T.reshape(H, D, C).transpose(1, 0, 2).reshape(C, C))
    wo3t = np.ascontiguousarray(
        wo3.T.reshape(H, D, C).transpose(1, 0, 2).reshape(C, C))

    in_maps = []
    for i in range(NCORES):
        sl = slice(i * nloc, (i + 1) * nloc)
        etc = np.ascontiguousarray(e_m[sl].reshape(nloc * Kn, Cin).T)
        xtc = np.ascontiguousarray(xf[sl].T)
        mc = np.ascontiguousarray(
            np.broadcast_to(Kn - cnt[sl], (C, nloc)).astype(np.float32))
        in_maps.append({
            "et": etc, "xt": xtc,
            "wqt": wqt, "wkt": wkt, "wvt": wvt, "hrep": hrep,
            "wost": wost, "wo3t": wo3t, "mcorr": mc,
        })
    return in_maps, nloc


def assemble_output(results, B, N):
    BN = B * N
    nloc = BN // NCORES
    outf = np.empty((BN, C), np.float32)
    for i, r in enumerate(results):
        outf[i * nloc:(i + 1) * nloc] = r["out"].T
    return outf.reshape(B, N, C)


def kernel(h_X, h_E, mask_attn, W_Q, W_K, W_V, W_O):
    in_maps, nloc = prep_inputs(h_X, h_E, mask_attn, W_Q, W_K, W_V, W_O)
    nc = _build_nc(nloc)
    res = run_bass_kernel_spmd(nc, in_maps, core_ids=list(range(NCORES)))
    B, N = h_X.shape[0], h_X.shape[1]
    return assemble_output(res.results, B, N)



# revision 5
# speedup vs baseline: 1.6630x; 1.6630x over previous
"""NeighborAttention (B=4, N=4096, K=32, C=128, H=4) on 8 Trainium2 cores.

Data-parallel over the flattened (B*N) node axis; weights replicated.
Channel-major layout [row (4d+h), node-major free].  All heavy tensors are
bf16; matmuls run at 1 cycle/row (4x the fp32 baseline).

Per 1024-col piece (32 nodes):
  KT   = WK' @ ET            (PE, 2 x 512-col matmuls -> 2-bank PSUM)
  prod = KT * bcast_j(QT)    (DVE 1x: fp32 PSUM operand)
  srep = Hrep @ prod         (PE)   head-summed scores, replicated over d
  e    = exp(srep)           (ACT -> bf16 SBUF; no max-subtraction needed)
  VT   = WV' @ ET            (PE)
  v    = copy(VT)            (ACT -> bf16 SBUF; enables 2x DVE below)
  uv   = e * v               (DVE 2x)
Per 256-node chunk: pairwise bf16 trees on DVE
  usum = sum_j uv, umax = max_j uv, z = sum_j e
Epilogue: z -= (K - cnt) host correction (masked slots contribute exp(0)=1),
  rz = 1/z (ACT), out = (WO_mean+WO_sum)' @ (usum*rz) + WO_max' @ (umax*rz).
attn sums to exactly 1, so aggr_mean == aggr_sum and the W_O blocks fold.
"""
import numpy as np
import ml_dtypes
import concourse.bass as bass
import concourse.bacc as bacc
import concourse.mybir as mybir
from concourse import tile
from concourse.bass_utils import run_bass_kernel_spmd

F32 = mybir.dt.float32
BF16 = mybir.dt.bfloat16
ALU = mybir.AluOpType
AF = mybir.ActivationFunctionType

K = 32
C = 128
H = 4
D = 32
NCORES = 8

CHUNK_N = 256
CHUNK_COLS = CHUNK_N * K        # 8192
PIECE = 1024                    # cols per DVE/ACT instruction (32 nodes)
MM = 512                        # cols per matmul (one PSUM bank)

_NC_CACHE = {}


def _tree(nc, src, n_nodes, w0, out_f32, tmps, op):
    """Pairwise reduce src [C, n_nodes*w0] over the w0 window -> out_f32."""
    cur = src
    w = w0
    for t in tmps:
        w //= 2
        a = cur[:].rearrange("p (n j) -> p n j", j=2 * w)
        nc.vector.tensor_tensor(
            t[:].rearrange("p (n j) -> p n j", j=w),
            a[:, :, 0:w], a[:, :, w:2 * w], op=op)
        cur = t
    a = cur[:].rearrange("p (n j) -> p n j", j=2)
    nc.vector.tensor_tensor(
        out_f32.unsqueeze(2), a[:, :, 0:1], a[:, :, 1:2], op=op)


def build_nc(nloc):
    if nloc in _NC_CACHE:
        return _NC_CACHE[nloc]
    assert nloc % CHUNK_N == 0
    nchunks = nloc // CHUNK_N

    nc = bacc.Bacc()
    et = nc.dram_tensor("et", [C, nloc * K], BF16, kind="ExternalInput")
    xt = nc.dram_tensor("xt", [C, nloc], BF16, kind="ExternalInput")
    wqt = nc.dram_tensor("wqt", [C, C], BF16, kind="ExternalInput")
    wkt = nc.dram_tensor("wkt", [C, C], BF16, kind="ExternalInput")
    wvt = nc.dram_tensor("wvt", [C, C], BF16, kind="ExternalInput")
    hrep = nc.dram_tensor("hrep", [C, C], BF16, kind="ExternalInput")
    wost = nc.dram_tensor("wost", [C, C], BF16, kind="ExternalInput")
    wo3t = nc.dram_tensor("wo3t", [C, C], BF16, kind="ExternalInput")
    mcorr = nc.dram_tensor("mcorr", [C, nloc], BF16, kind="ExternalInput")
    out = nc.dram_tensor("out", [C, nloc], F32, kind="ExternalOutput")

    with tile.TileContext(nc) as tc:
        with tc.tile_pool(name="wts", bufs=1) as wpool, \
             tc.tile_pool(name="xin", bufs=1) as xpool, \
             tc.tile_pool(name="etp", bufs=2) as etpool, \
             tc.tile_pool(name="qp", bufs=2) as qpool, \
             tc.tile_pool(name="pp", bufs=2) as ppool, \
             tc.tile_pool(name="vp", bufs=2) as vpool, \
             tc.tile_pool(name="ep", bufs=2) as epool, \
             tc.tile_pool(name="uvp", bufs=2) as uvpool, \
             tc.tile_pool(name="tp", bufs=1) as tpool, \
             tc.tile_pool(name="acc", bufs=1) as accp, \
             tc.tile_pool(name="epi", bufs=1) as epip, \
             tc.tile_pool(name="outp", bufs=1) as outp, \
             tc.tile_pool(name="pkt", bufs=2, space="PSUM") as pkt, \
             tc.tile_pool(name="pvt", bufs=1, space="PSUM") as pvt, \
             tc.tile_pool(name="psr", bufs=1, space="PSUM") as psr:

            w_q = wpool.tile([C, C], BF16, tag="wq")
            w_k = wpool.tile([C, C], BF16, tag="wk")
            w_v = wpool.tile([C, C], BF16, tag="wv")
            w_h = wpool.tile([C, C], BF16, tag="wh")
            w_os = wpool.tile([C, C], BF16, tag="wos")
            w_o3 = wpool.tile([C, C], BF16, tag="wo3")
            nc.sync.dma_start(w_q[:], wqt[:])
            nc.sync.dma_start(w_k[:], wkt[:])
            nc.sync.dma_start(w_v[:], wvt[:])
            nc.sync.dma_start(w_h[:], hrep[:])
            nc.sync.dma_start(w_os[:], wost[:])
            nc.sync.dma_start(w_o3[:], wo3t[:])

            xt_sb = xpool.tile([C, nloc], BF16, tag="xt")
            nc.sync.dma_start(xt_sb[:], xt[:])
            mc_sb = xpool.tile([C, nloc], BF16, tag="mc")
            nc.sync.dma_start(mc_sb[:], mcorr[:])

            usum_c = accp.tile([C, nloc], F32, tag="usum")
            umax_c = accp.tile([C, nloc], F32, tag="umax")
            z_c = accp.tile([C, nloc], F32, tag="zc")

            for ch in range(nchunks):
                n0 = ch * CHUNK_N
                c0 = ch * CHUNK_COLS

                et_sb = etpool.tile([C, CHUNK_COLS], BF16, tag="et")
                nc.sync.dma_start(et_sb[:], et[:, c0:c0 + CHUNK_COLS])

                q_ps = pkt.tile([C, PIECE], F32, tag="kt")
                nc.tensor.matmul(q_ps[:, :CHUNK_N], w_q[:],
                                 xt_sb[:, n0:n0 + CHUNK_N],
                                 start=True, stop=True)
                q_sb = qpool.tile([C, CHUNK_N], BF16, tag="q")
                nc.vector.tensor_copy(q_sb[:], q_ps[:, :CHUNK_N])

                e_ch = epool.tile([C, CHUNK_COLS], BF16, tag="e")
                uv_ch = uvpool.tile([C, CHUNK_COLS], BF16, tag="uv")

                for p0 in range(0, CHUNK_COLS, PIECE):
                    pn0 = p0 // K          # first node of piece (in chunk)
                    pnn = PIECE // K       # nodes per piece

                    kt_ps = pkt.tile([C, PIECE], F32, tag="kt")
                    nc.tensor.matmul(kt_ps[:, :MM], w_k[:],
                                     et_sb[:, p0:p0 + MM],
                                     start=True, stop=True)
                    nc.tensor.matmul(kt_ps[:, MM:], w_k[:],
                                     et_sb[:, p0 + MM:p0 + PIECE],
                                     start=True, stop=True)

                    prod = ppool.tile([C, PIECE], BF16, tag="prod")
                    qb = q_sb[:, pn0:pn0 + pnn].unsqueeze(2).broadcast_to(
                        (C, pnn, K))
                    nc.vector.tensor_mul(
                        prod[:].rearrange("p (n j) -> p n j", j=K),
                        kt_ps[:].rearrange("p (n j) -> p n j", j=K), qb)

                    sr_ps = psr.tile([C, PIECE], F32, tag="sr")
                    nc.tensor.matmul(sr_ps[:, :MM], w_h[:], prod[:, :MM],
                                     start=True, stop=True)
                    nc.tensor.matmul(sr_ps[:, MM:], w_h[:], prod[:, MM:],
                                     start=True, stop=True)
                    nc.scalar.activation(e_ch[:, p0:p0 + PIECE], sr_ps[:],
                                         AF.Exp)

                    vt_ps = pvt.tile([C, PIECE], F32, tag="vt")
                    nc.tensor.matmul(vt_ps[:, :MM], w_v[:],
                                     et_sb[:, p0:p0 + MM],
                                     start=True, stop=True)
                    nc.tensor.matmul(vt_ps[:, MM:], w_v[:],
                                     et_sb[:, p0 + MM:p0 + PIECE],
                                     start=True, stop=True)
                    v_sb = vpool.tile([C, PIECE], BF16, tag="v")
                    nc.scalar.activation(v_sb[:], vt_ps[:], AF.Copy)

                    nc.vector.tensor_mul(uv_ch[:, p0:p0 + PIECE],
                                         e_ch[:, p0:p0 + PIECE], v_sb[:])

                t16 = tpool.tile([C, CHUNK_N * 16], BF16, tag="t16")
                t8 = tpool.tile([C, CHUNK_N * 8], BF16, tag="t8")
                t4 = tpool.tile([C, CHUNK_N * 4], BF16, tag="t4")
                t2 = tpool.tile([C, CHUNK_N * 2], BF16, tag="t2")
                _tree(nc, uv_ch, CHUNK_N, K,
                      usum_c[:, n0:n0 + CHUNK_N], [t16, t8, t4, t2], ALU.add)
                m16 = tpool.tile([C, CHUNK_N * 16], BF16, tag="t16")
                m8 = tpool.tile([C, CHUNK_N * 8], BF16, tag="t8")
                m4 = tpool.tile([C, CHUNK_N * 4], BF16, tag="t4")
                m2 = tpool.tile([C, CHUNK_N * 2], BF16, tag="t2")
                _tree(nc, uv_ch, CHUNK_N, K,
                      umax_c[:, n0:n0 + CHUNK_N], [m16, m8, m4, m2], ALU.max)
                z16 = tpool.tile([C, CHUNK_N * 16], BF16, tag="t16")
                z8 = tpool.tile([C, CHUNK_N * 8], BF16, tag="t8")
                z4 = tpool.tile([C, CHUNK_N * 4], BF16, tag="t4")
                z2 = tpool.tile([C, CHUNK_N * 2], BF16, tag="t2")
                _tree(nc, e_ch, CHUNK_N, K,
                      z_c[:, n0:n0 + CHUNK_N], [z16, z8, z4, z2], ALU.add)

            # epilogue
            ztmp = epip.tile([C, nloc], F32, tag="ztmp")
            nc.vector.tensor_sub(ztmp[:], z_c[:], mc_sb[:])
            # fully-masked nodes: usum/umax rows are exactly 0; any finite
            # 1/z gives the correct 0 output — just avoid inf*0.
            nc.vector.tensor_scalar_max(ztmp[:], ztmp[:], 1e-20)
            # 1/z = exp(-ln(z)): Ln and Exp share one ACT table set, so no
            # table reload; bass blocks the Reciprocal ACT function.
            lnz = epip.tile([C, nloc], F32, tag="lnz")
            nc.scalar.activation(lnz[:], ztmp[:], AF.Ln)
            rz = epip.tile([C, nloc], F32, tag="rz")
            nc.scalar.activation(rz[:], lnz[:], AF.Exp, scale=-1.0)

            wsn = epip.tile([C, nloc], BF16, tag="wsn")
            nc.vector.tensor_mul(wsn[:], usum_c[:], rz[:])
            mxn = epip.tile([C, nloc], BF16, tag="mxn")
            nc.vector.tensor_mul(mxn[:], umax_c[:], rz[:])

            out_sb = outp.tile([C, nloc], F32, tag="osb")
            for b0 in range(0, nloc, MM):
                o_ps = psr.tile([C, PIECE], F32, tag="sr")
                nc.tensor.matmul(o_ps[:, :MM], w_os[:], wsn[:, b0:b0 + MM],
                                 start=True, stop=False)
                nc.tensor.matmul(o_ps[:, :MM], w_o3[:], mxn[:, b0:b0 + MM],
                                 start=False, stop=True)
                nc.scalar.activation(out_sb[:, b0:b0 + MM], o_ps[:, :MM],
                                     AF.Copy)
            nc.sync.dma_start(out[:], out_sb[:])

    nc.compile()
    _NC_CACHE[nloc] = nc
    return nc


def _perm_dh(w):
    """[(h*32+d), cin] -> [cin, (4d+h)] in bf16"""
    wt = np.asarray(w, dtype=np.float32).reshape(H, D, -1)
    return np.ascontiguousarray(
        np.transpose(wt, (2, 1, 0)).reshape(-1, H * D)).astype(
            ml_dtypes.bfloat16)


def prep_inputs(h_X, h_E, mask_attn, W_Q, W_K, W_V, W_O):
    h_X = np.asarray(h_X, dtype=np.float32)
    h_E = np.asarray(h_E, dtype=np.float32)
    mask_attn = np.asarray(mask_attn)
    W_Q = np.asarray(W_Q, dtype=np.float32)
    W_K = np.asarray(W_K, dtype=np.float32)
    W_V = np.asarray(W_V, dtype=np.float32)
    W_O = np.asarray(W_O, dtype=np.float32)

    B, N, Kn, Cin = h_E.shape
    BN = B * N
    nloc = BN // NCORES

    maskf = mask_attn.astype(np.float32)
    e_m = (h_E * maskf[..., None]).reshape(BN, Kn, Cin)
    xf = h_X.reshape(BN, -1)
    cnt = maskf.reshape(BN, Kn).sum(axis=1)

    wqt = _perm_dh(W_Q / np.sqrt(D))
    wkt = _perm_dh(W_K)
    wvt = _perm_dh(W_V)

    idx = np.arange(C)
    hh = idx % H
    hrep = (hh[:, None] == hh[None, :]).astype(ml_dtypes.bfloat16)

    wos = W_O[:, :C] + W_O[:, C:2 * C]
    wo3 = W_O[:, 2 * C:]
    wost = np.ascontiguousarray(
        wos.T.reshape(H, D, C).transpose(1, 0, 2).reshape(C, C)).astype(
            ml_dtypes.bfloat16)
    wo3t = np.ascontiguousarray(
        wo3.T.reshape(H, D, C).transpose(1, 0, 2).reshape(C, C)).astype(
            ml_dtypes.bfloat16)

    in_maps = []
    for i in range(NCORES):
        sl = slice(i * nloc, (i + 1) * nloc)
        etc = np.ascontiguousarray(
            e_m[sl].reshape(nloc * Kn, Cin).T).astype(ml_dtypes.bfloat16)
        xtc = np.ascontiguousarray(xf[sl].T).astype(ml_dtypes.bfloat16)
        mc = np.ascontiguousarray(
            np.broadcast_to(Kn - cnt[sl], (C, nloc))).astype(
                ml_dtypes.bfloat16)
        in_maps.append({
            "et": etc, "xt": xtc,
            "wqt": wqt, "wkt": wkt, "wvt": wvt, "hrep": hrep,
            "wost": wost, "wo3t": wo3t, "mcorr": mc,
        })
    return in_maps, nloc


def assemble_output(results, B, N):
    BN = B * N
    nloc = BN // NCORES
    outf = np.empty((BN, C), np.float32)
    for i, r in enumerate(results):
        outf[i * nloc:(i + 1) * nloc] = r["out"].T
    return outf.reshape(B, N, C)


def kernel(h_X, h_E, mask_attn, W_Q, W_K, W_V, W_O):
    in_maps, nloc = prep_inputs(h_X, h_E, mask_attn, W_Q, W_K, W_V, W_O)
    nc = build_nc(nloc)
    res = run_bass_kernel_spmd(nc, in_maps, core_ids=list(range(NCORES)))
    B, N = h_X.shape[0], h_X.shape[1]
    return assemble_output(res.results, B, N)


# revision 9
# speedup vs baseline: 2.5053x; 1.5065x over previous
"""NeighborAttention (B=4, N=4096, K=32, C=128, H=4) on 8 Trainium2 cores.

Data-parallel over the flattened (B*N) node axis; weights replicated.
Channel-major layout [row (4d+h), node-major free].  All heavy tensors are
bf16; matmuls run at 1 cycle/row.

Mask-aware bucketing: attention is permutation-invariant over the K
neighbors, and masked neighbors are zeroed.  The host packs each node's
unmasked neighbors first, rounds the count up to a bucket width
Kb in {8,12,16,20,24,28,32}, sorts nodes by bucket, and deals them
round-robin to the 8 cores so every core sees identical bucket counts
(padded by at most 7 dummy nodes).  Since E[cnt]=16, this drops ~45% of
all columns from every engine.  Padded slots have et=0, so they score 0
and contribute exp(0)=1 to the softmax denominator; the host sends the
per-node count correction (Kb - cnt) to subtract.

Per piece (<=1024 cols):
  KT   = WK' @ ET            (PE, 512-col matmuls -> 2-bank PSUM)
  prod = KT * bcast_j(QT)    (DVE 1x: fp32 PSUM operand)
  srep = Hrep @ prod         (PE)   head-summed scores, replicated over d
  e    = exp(srep)           (ACT -> bf16 SBUF; shift-invariance makes
                              max-subtraction unnecessary at these scales)
  VT   = WV' @ ET            (PE)
  v    = copy(VT)            (ACT -> bf16 SBUF; enables 2x DVE below)
  uv   = e * v               (DVE 2x)
Per chunk (<=8192 cols): pairwise bf16 trees on DVE
  usum = sum_j uv, umax = max_j uv, z = sum_j e
Epilogue: z -= (Kb - cnt), rz = exp(-ln(z)) on ACT,
  out = (WO_mean+WO_sum)' @ (usum*rz) + WO_max' @ (umax*rz).
attn sums to exactly 1, so aggr_mean == aggr_sum and the W_O blocks fold.
"""
import numpy as np
import ml_dtypes
import concourse.bass as bass
import concourse.bacc as bacc
import concourse.mybir as mybir
from concourse import tile
from concourse.bass_utils import run_bass_kernel_spmd

F32 = mybir.dt.float32
BF16 = mybir.dt.bfloat16
ALU = mybir.AluOpType
AF = mybir.ActivationFunctionType

K = 32
C = 128
H = 4
D = 32
NCORES = 8

BUCKETS = (8, 12, 16, 20, 24, 28, 32)
CHUNK_COLS = 8192
PIECE_COLS = 1024
MM = 512

_NC_CACHE = {}


def _tree_seg(nc, tmps, src, nn, w, out_f32, op):
    """Pairwise-reduce src [C, nn*w] windows of w -> out_f32 [C, nn]."""
    cur = src[:, :nn * w].rearrange("p (n j) -> p n j", j=w)
    li = 0
    while w > 2:
        h, odd = w // 2, w % 2
        wout = h + odd
        tt = tmps[li % len(tmps)]
        assert tt.shape[1] >= nn * wout, (nn, wout)
        t = tt[:, :nn * wout].rearrange("p (n j) -> p n j", j=wout)
        nc.vector.tensor_tensor(t[:, :, 0:h], cur[:, :, 0:h],
                                cur[:, :, h:2 * h], op=op)
        if odd:
            nc.vector.tensor_copy(t[:, :, h:h + 1], cur[:, :, 2 * h:2 * h + 1])
        cur = t
        w = wout
        li += 1
    nc.vector.tensor_tensor(out_f32.unsqueeze(2), cur[:, :, 0:1],
                            cur[:, :, 1:2], op=op)


def build_nc(nloc_pad, segments):
    """segments: tuple of (Kb, n_nodes) with sum(n_nodes) == nloc_pad."""
    key = (nloc_pad, segments)
    if key in _NC_CACHE:
        return _NC_CACHE[key]
    total_cols = sum(kb * nn for kb, nn in segments)

    nc = bacc.Bacc()
    et = nc.dram_tensor("et", [C, total_cols], BF16, kind="ExternalInput")
    xt = nc.dram_tensor("xt", [C, nloc_pad], BF16, kind="ExternalInput")
    wqt = nc.dram_tensor("wqt", [C, C], BF16, kind="ExternalInput")
    wkt = nc.dram_tensor("wkt", [C, C], BF16, kind="ExternalInput")
    wvt = nc.dram_tensor("wvt", [C, C], BF16, kind="ExternalInput")
    hrep = nc.dram_tensor("hrep", [C, C], BF16, kind="ExternalInput")
    wost = nc.dram_tensor("wost", [C, C], BF16, kind="ExternalInput")
    wo3t = nc.dram_tensor("wo3t", [C, C], BF16, kind="ExternalInput")
    mcorr = nc.dram_tensor("mcorr", [C, nloc_pad], BF16, kind="ExternalInput")
    out = nc.dram_tensor("out", [C, nloc_pad], F32, kind="ExternalOutput")

    with tile.TileContext(nc) as tc:
        with tc.tile_pool(name="wts", bufs=1) as wpool, \
             tc.tile_pool(name="xin", bufs=1) as xpool, \
             tc.tile_pool(name="etp", bufs=2) as etpool, \
             tc.tile_pool(name="qp", bufs=2) as qpool, \
             tc.tile_pool(name="pp", bufs=2) as ppool, \
             tc.tile_pool(name="vp", bufs=2) as vpool, \
             tc.tile_pool(name="ep", bufs=2) as epool, \
             tc.tile_pool(name="uvp", bufs=2) as uvpool, \
             tc.tile_pool(name="tp", bufs=1) as tpool, \
             tc.tile_pool(name="acc", bufs=1) as accp, \
             tc.tile_pool(name="epi", bufs=1) as epip, \
             tc.tile_pool(name="outp", bufs=1) as outp, \
             tc.tile_pool(name="pkt", bufs=2, space="PSUM") as pkt, \
             tc.tile_pool(name="pvt", bufs=1, space="PSUM") as pvt, \
             tc.tile_pool(name="psr", bufs=1, space="PSUM") as psr:

            w_q = wpool.tile([C, C], BF16, tag="wq")
            w_k = wpool.tile([C, C], BF16, tag="wk")
            w_v = wpool.tile([C, C], BF16, tag="wv")
            w_h = wpool.tile([C, C], BF16, tag="wh")
            w_os = wpool.tile([C, C], BF16, tag="wos")
            w_o3 = wpool.tile([C, C], BF16, tag="wo3")
            nc.sync.dma_start(w_q[:], wqt[:])
            nc.sync.dma_start(w_k[:], wkt[:])
            nc.sync.dma_start(w_v[:], wvt[:])
            nc.sync.dma_start(w_h[:], hrep[:])
            nc.sync.dma_start(w_os[:], wost[:])
            nc.sync.dma_start(w_o3[:], wo3t[:])

            xt_sb = xpool.tile([C, nloc_pad], BF16, tag="xt")
            nc.sync.dma_start(xt_sb[:], xt[:])
            mc_sb = xpool.tile([C, nloc_pad], BF16, tag="mc")
            nc.sync.dma_start(mc_sb[:], mcorr[:])

            usum_c = accp.tile([C, nloc_pad], F32, tag="usum")
            umax_c = accp.tile([C, nloc_pad], F32, tag="umax")
            z_c = accp.tile([C, nloc_pad], F32, tag="zc")

            tr0 = tpool.tile([C, 4096], BF16, tag="t0")
            tr1 = tpool.tile([C, 2048], BF16, tag="t1")
            tr2 = tpool.tile([C, 2048], BF16, tag="t2")
            tmps = [tr0, tr1, tr2]

            node_off = 0
            col_off = 0
            for kb, seg_nodes in segments:
                chunk_n = CHUNK_COLS // kb
                piece_n = PIECE_COLS // kb
                for ch0 in range(0, seg_nodes, chunk_n):
                    nn = min(chunk_n, seg_nodes - ch0)
                    ccols = nn * kb
                    n0 = node_off + ch0
                    c0 = col_off + ch0 * kb

                    et_sb = etpool.tile([C, CHUNK_COLS], BF16, tag="et")
                    nc.sync.dma_start(et_sb[:, :ccols], et[:, c0:c0 + ccols])

                    q_ps = pkt.tile([C, PIECE_COLS], F32, tag="kt")
                    nc.tensor.matmul(q_ps[:, :nn], w_q[:],
                                     xt_sb[:, n0:n0 + nn],
                                     start=True, stop=True)
                    q_sb = qpool.tile([C, 1024], BF16, tag="q")
                    nc.vector.tensor_copy(q_sb[:, :nn], q_ps[:, :nn])

                    e_ch = epool.tile([C, CHUNK_COLS], BF16, tag="e")
                    uv_ch = uvpool.tile([C, CHUNK_COLS], BF16, tag="uv")

                    for p0 in range(0, nn, piece_n):
                        pnn = min(piece_n, nn - p0)
                        pc = pnn * kb          # cols in piece
                        pc0 = p0 * kb          # col offset in chunk

                        kt_ps = pkt.tile([C, PIECE_COLS], F32, tag="kt")
                        s = min(MM, pc)
                        nc.tensor.matmul(kt_ps[:, :s], w_k[:],
                                         et_sb[:, pc0:pc0 + s],
                                         start=True, stop=True)
                        if pc > MM:
                            nc.tensor.matmul(kt_ps[:, MM:pc], w_k[:],
                                             et_sb[:, pc0 + MM:pc0 + pc],
                                             start=True, stop=True)

                        prod = ppool.tile([C, PIECE_COLS], BF16, tag="prod")
                        qb = q_sb[:, p0:p0 + pnn].unsqueeze(2).broadcast_to(
                            (C, pnn, kb))
                        nc.vector.tensor_mul(
                            prod[:, :pc].rearrange("p (n j) -> p n j", j=kb),
                            kt_ps[:, :pc].rearrange("p (n j) -> p n j", j=kb),
                            qb)

                        sr_ps = psr.tile([C, PIECE_COLS], F32, tag="sr")
                        nc.tensor.matmul(sr_ps[:, :s], w_h[:], prod[:, :s],
                                         start=True, stop=True)
                        if pc > MM:
                            nc.tensor.matmul(sr_ps[:, MM:pc], w_h[:],
                                             prod[:, MM:pc],
                                             start=True, stop=True)
                        nc.scalar.activation(e_ch[:, pc0:pc0 + pc],
                                             sr_ps[:, :pc], AF.Exp)

                        vt_ps = pvt.tile([C, PIECE_COLS], F32, tag="vt")
                        nc.tensor.matmul(vt_ps[:, :s], w_v[:],
                                         et_sb[:, pc0:pc0 + s],
                                         start=True, stop=True)
                        if pc > MM:
                            nc.tensor.matmul(vt_ps[:, MM:pc], w_v[:],
                                             et_sb[:, pc0 + MM:pc0 + pc],
                                             start=True, stop=True)
                        v_sb = vpool.tile([C, PIECE_COLS], BF16, tag="v")
                        nc.scalar.activation(v_sb[:, :pc], vt_ps[:, :pc],
                                             AF.Copy)

                        nc.vector.tensor_mul(uv_ch[:, pc0:pc0 + pc],
                                             e_ch[:, pc0:pc0 + pc],
                                             v_sb[:, :pc])

                    _tree_seg(nc, tmps, uv_ch, nn, kb,
                              usum_c[:, n0:n0 + nn], ALU.add)
                    _tree_seg(nc, tmps, uv_ch, nn, kb,
                              umax_c[:, n0:n0 + nn], ALU.max)
                    _tree_seg(nc, tmps, e_ch, nn, kb,
                              z_c[:, n0:n0 + nn], ALU.add)

                node_off += seg_nodes
                col_off += seg_nodes * kb

            # epilogue
            ztmp = epip.tile([C, nloc_pad], F32, tag="ztmp")
            nc.vector.tensor_sub(ztmp[:], z_c[:], mc_sb[:])
            # fully-masked nodes: usum/umax rows are exactly 0; any finite
            # 1/z gives the correct 0 output — just avoid inf*0.
            nc.vector.tensor_scalar_max(ztmp[:], ztmp[:], 1e-20)
            # 1/z = exp(-ln(z)): Ln and Exp share one ACT table set; bass
            # blocks the Reciprocal ACT function for accuracy.
            lnz = epip.tile([C, nloc_pad], F32, tag="lnz")
            nc.scalar.activation(lnz[:], ztmp[:], AF.Ln)
            rz = epip.tile([C, nloc_pad], F32, tag="rz")
            nc.scalar.activation(rz[:], lnz[:], AF.Exp, scale=-1.0)

            wsn = epip.tile([C, nloc_pad], BF16, tag="wsn")
            nc.vector.tensor_mul(wsn[:], usum_c[:], rz[:])
            mxn = epip.tile([C, nloc_pad], BF16, tag="mxn")
            nc.vector.tensor_mul(mxn[:], umax_c[:], rz[:])

            out_sb = outp.tile([C, nloc_pad], F32, tag="osb")
            for b0 in range(0, nloc_pad, MM):
                ob = min(MM, nloc_pad - b0)
                o_ps = psr.tile([C, PIECE_COLS], F32, tag="sr")
                nc.tensor.matmul(o_ps[:, :ob], w_os[:], wsn[:, b0:b0 + ob],
                                 start=True, stop=False)
                nc.tensor.matmul(o_ps[:, :ob], w_o3[:], mxn[:, b0:b0 + ob],
                                 start=False, stop=True)
                nc.scalar.activation(out_sb[:, b0:b0 + ob], o_ps[:, :ob],
                                     AF.Copy)
            nc.sync.dma_start(out[:], out_sb[:])

    nc.compile()
    _NC_CACHE[key] = nc
    return nc


def _perm_dh(w):
    """[(h*32+d), cin] -> [cin, (4d+h)] in bf16"""
    wt = np.asarray(w, dtype=np.float32).reshape(H, D, -1)
    return np.ascontiguousarray(
        np.transpose(wt, (2, 1, 0)).reshape(-1, H * D)).astype(
            ml_dtypes.bfloat16)


def prep_inputs(h_X, h_E, mask_attn, W_Q, W_K, W_V, W_O):
    h_X = np.asarray(h_X, dtype=np.float32)
    h_E = np.asarray(h_E, dtype=np.float32)
    mask_attn = np.asarray(mask_attn)
    W_Q = np.asarray(W_Q, dtype=np.float32)
    W_K = np.asarray(W_K, dtype=np.float32)
    W_V = np.asarray(W_V, dtype=np.float32)
    W_O = np.asarray(W_O, dtype=np.float32)

    B, N, Kn, Cin = h_E.shape
    BN = B * N

    maskf = mask_attn.astype(np.float32).reshape(BN, Kn)
    ef = h_E.reshape(BN, Kn, Cin)
    xf = h_X.reshape(BN, -1)
    cnt = maskf.sum(axis=1).astype(np.int64)

    # bucket per node, neighbor packing order (unmasked first, stable)
    barr = np.asarray(BUCKETS)
    bidx = np.searchsorted(barr, cnt)          # index of smallest Kb >= cnt
    perm_j = np.argsort(-maskf, axis=1, kind="stable")

    # sort nodes by bucket (stable), deal round-robin to cores
    order = np.argsort(bidx, kind="stable")
    core_ids = [order[i::NCORES] for i in range(NCORES)]
    # per-core per-bucket counts; pad to max over cores
    nb = np.zeros((NCORES, len(BUCKETS)), np.int64)
    for i in range(NCORES):
        nb[i] = np.bincount(bidx[core_ids[i]], minlength=len(BUCKETS))
    nb_max = nb.max(axis=0)
    segments = tuple((int(barr[b]), int(nb_max[b]))
                     for b in range(len(BUCKETS)) if nb_max[b] > 0)
    nloc_pad = int(nb_max.sum())
    total_cols = sum(kb * nn for kb, nn in segments)

    wqt = _perm_dh(W_Q / np.sqrt(D))
    wkt = _perm_dh(W_K)
    wvt = _perm_dh(W_V)

    idx = np.arange(C)
    hh = idx % H
    hrep = (hh[:, None] == hh[None, :]).astype(ml_dtypes.bfloat16)

    wos = W_O[:, :C] + W_O[:, C:2 * C]
    wo3 = W_O[:, 2 * C:]
    wost = np.ascontiguousarray(
        wos.T.reshape(H, D, C).transpose(1, 0, 2).reshape(C, C)).astype(
            ml_dtypes.bfloat16)
    wo3t = np.ascontiguousarray(
        wo3.T.reshape(H, D, C).transpose(1, 0, 2).reshape(C, C)).astype(
            ml_dtypes.bfloat16)

    in_maps = []
    ids_padded_all = []
    for i in range(NCORES):
        ids = core_ids[i]
        etc = np.zeros((C, total_cols), ml_dtypes.bfloat16)
        xtc = np.zeros((C, nloc_pad), ml_dtypes.bfloat16)
        mcc = np.zeros((C, nloc_pad), ml_dtypes.bfloat16)
        ids_padded = np.full(nloc_pad, -1, np.int64)
        no = 0
        co = 0
        for b, (kb, nn_seg) in zip(
                [b for b in range(len(BUCKETS)) if nb_max[b] > 0], segments):
            sel = ids[bidx[ids] == b]
            nsel = len(sel)
            if nsel:
                pj = perm_j[sel][:, :kb]                      # [nsel, kb]
                g = np.take_along_axis(ef[sel], pj[:, :, None], axis=1)
                gm = np.take_along_axis(maskf[sel], pj, axis=1)
                g = g * gm[:, :, None]                        # [nsel, kb, C]
                etc[:, co:co + nsel * kb] = g.reshape(
                    nsel * kb, Cin).T.astype(ml_dtypes.bfloat16)
                xtc[:, no:no + nsel] = xf[sel].T.astype(ml_dtypes.bfloat16)
                mcc[:, no:no + nsel] = np.broadcast_to(
                    kb - cnt[sel], (C, nsel)).astype(ml_dtypes.bfloat16)
                ids_padded[no:no + nsel] = sel
            # padded dummy nodes: et/x zero, correction = kb so z-mc = 0
            if nn_seg > nsel:
                mcc[:, no + nsel:no + nn_seg] = np.float32(kb)
            no += nn_seg
            co += nn_seg * kb
        ids_padded_all.append(ids_padded)
        in_maps.append({
            "et": etc, "xt": xtc,
            "wqt": wqt, "wkt": wkt, "wvt": wvt, "hrep": hrep,
            "wost": wost, "wo3t": wo3t, "mcorr": mcc,
        })
    meta = {"segments": segments, "nloc_pad": nloc_pad,
            "ids_padded": ids_padded_all}
    return in_maps, meta


def assemble_output(results, B, N, meta):
    BN = B * N
    outf = np.empty((BN, C), np.float32)
    for i, r in enumerate(results):
        ids = meta["ids_padded"][i]
        valid = ids >= 0
        outf[ids[valid]] = r["out"].T[valid]
    return outf.reshape(B, N, C)


def kernel(h_X, h_E, mask_attn, W_Q, W_K, W_V, W_O):
    in_maps, meta = prep_inputs(h_X, h_E, mask_attn, W_Q, W_K, W_V, W_O)
    nc = build_nc(meta["nloc_pad"], meta["segments"])
    res = run_bass_kernel_spmd(nc, in_maps, core_ids=list(range(NCORES)))
    B, N = h_X.shape[0], h_X.shape[1]
    return assemble_output(res.results, B, N, meta)
